# revision 19
# baseline (speedup 1.0000x reference)
"""Trainium2 Bass kernel for LUT-based int8-quantized 3x3 conv (ApproxTorch baseline).

Problem: y = conv2d(quant(x), quant(w)) summed via a 256x256 LUT of int8
products, rescaled by (T_f/127)*(T_w/127) + bias, where T_f/T_w are EMA
thresholds updated with the *global* absmax of x / w before the conv.

The LUT staged by setup_inputs() is the exact signed-product table
lut[a+128, b+128] = a*b, so the LUT-gather-sum is an integer matmul; int8
values are exact in bf16 and accumulate exactly in fp32 PSUM, so the PE
array reproduces the reference. We verify the product-table property on
the host and refuse to run otherwise.

Sharding: data-parallel over batch (B=8 -> 1 image/core). The global absmax
of x needs all 8 images on every core: a bf16 replica of the full batch is
loaded per core (absmax only; bf16 rounding of the max costs ~6e-4 output
rel err vs the 2e-2 gate) and the own image in fp32 for exact quantization.
This avoids the ~20us mesh-AllReduce latency floor entirely.

Measured HW facts driving the layout (from NTFF traces):
 - DMA queues are descriptor-dispatch-bound (~27ns/desc scalar HWDGE,
   ~35ns sync, ~42ns gpsimd SWDGE) -> minimize descriptor count: every
   tensor is loaded as [64, long-row] partition halves on its own queue.
 - DVE TENSOR_REDUCE is ~1.09 cyc/elem at any dtype; 16-bit elementwise
   TT/TS ops get the 2x mode (0.63) -> reduce the bf16 batch replica with
   a TT-max tree (halving each level), not a flat reduce.
 - 8-bit DVE ops get no fast mode, so fp8 only shrinks bytes (irrelevant
   when descriptor-bound) -> bf16 replica.

Layout:
 - xbw [128, 1291] f32 (5164B rows): cols 0:900 padded own image (top =
   rows 0:30 of the 30x30 pad-1 image, bottom = the same shifted one
   padded row, so one [128,14,28] moving AP feeds tap (kh,kw) from the
   top half and (kh+1,kw) from the bottom), col 904 bias, cols 907:1291
   wpair (pair groups g0-2 as top/bottom taps (0,g)/(1,g); kh=2 singles
   g3-5 in the top half).
 - xob8 [128, 3136] bf16 (6272B rows): the full batch, absmax only.

Queues: scalar: xob8 top, then xbw bottom; sync: xbw top; gpsimd: xob8
bottom. Outputs are partition-split across scalar+sync.

Pipeline: TT-max tree (top half early, bottom on landing) -> per-partition
max -> gpsimd partition all-reduce -> trec = am*(0.05/127) + 0.95*T/127 ->
qscale = 1/trec (DVE reciprocal) -> quantize via the 1.5*2^23 RNE trick
(ACT Copy(v*qs+MAGIC) -> DVE (t-MAGIC,max -128) -> (min 127) -> bf16) ->
12 matmuls (2 PSUM banks x 6 groups) -> out = psum*(trec_x*trec_w) + bias.
"""

import os
import sys

import numpy as np

for _p in ("/opt/trn_rl_repo", "/root/.axon_site", "/root/.axon_site/_ro/trn_rl_repo",
           "/root/.axon_site/_ro/pypackages"):
    if os.path.isdir(_p) and _p not in sys.path:
        sys.path.append(_p)

import ml_dtypes  # noqa: E402

from concourse import bacc, bass, bass_isa, mybir, tile  # noqa: E402
from concourse.bass_utils import run_bass_kernel_spmd  # noqa: E402

F32 = mybir.dt.float32
BF16 = mybir.dt.bfloat16
AX = mybir.AxisListType
OP = mybir.AluOpType
ACTF = mybir.ActivationFunctionType

N_CORES = 8
CIN = 64
COUT = 64
H = W = 28
P = H * W            # 784 pixels
PH = P // 2          # 392 per PSUM bank (14 output rows)
PAD = 30             # padded spatial edge
XB_F = 907           # padded image block: 900 image + 4 zeros + bias + 2 spare
W_F = 384            # wpair cols: 3 pair groups + 3 kh=2 singles (top half)
XBW_F = XB_F + W_F   # 1291
XO_F = 8 * CIN * P // 128  # 3136: the full batch as bf16
MAGIC = 12582912.0   # 1.5 * 2**23: fp32 add/sub round-to-nearest-even trick

# trec = absmax*(0.05/127) + 0.95*T_init/127; qscale = 1/trec
TREC_MUL = float(np.float32(0.05) / np.float32(127.0))
TREC_ADD_X = float(np.float32(0.95) * np.float32(3.0) / np.float32(127.0))
TREC_ADD_W = float(np.float32(0.95) * np.float32(0.3) / np.float32(127.0))


def _build():
    nc = bacc.Bacc(
        "TRN2",
        target_bir_lowering=False,
        debug=False,
        enable_asserts=True,
        num_devices=N_CORES,
    )
    xbw_d = nc.dram_tensor("xbw", [2 * CIN, XBW_F], F32, kind="ExternalInput")
    xob_d = nc.dram_tensor("xob", [128, XO_F], BF16, kind="ExternalInput")
    out_d = nc.dram_tensor("out", [COUT, P], F32, kind="ExternalOutput")

    with tile.TileContext(nc) as tc:
        with (
            tc.tile_pool(name="sbuf", bufs=1) as pool,
            tc.tile_pool(name="psum", bufs=1, space="PSUM") as psum,
        ):
            # ---- loads: partition-split halves, one per queue slot, to
            # minimize per-queue descriptor counts (the measured bound).
            xbw = pool.tile([2 * CIN, XBW_F], F32)
            xob = pool.tile([128, XO_F], BF16)
            warm = pool.tile([1, 8], BF16)
            nc.scalar.dma_start(out=warm[:], in_=xob_d[0:1, 0:8])
            nc.sync.dma_start(out=xob[0:64, :], in_=xob_d[0:64, :])
            nc.scalar.dma_start(out=xob[64:128, :], in_=xob_d[64:128, :])
            nc.sync.dma_start(out=xbw[0:CIN, :], in_=xbw_d[0:CIN, :])
            nc.scalar.dma_start(out=xbw[CIN:2 * CIN, :],
                                in_=xbw_d[CIN:2 * CIN, :])
            w_sb = xbw[:, XB_F:XBW_F]
            bias_sb = xbw[0:COUT, 904:905]

            # ---- absmax of the batch: bf16 TT-max tree (2x DVE mode).
            # Level 1 runs per partition half so the early half overlaps
            # the other half's DMA. |.| via abs into the tree's level 1.
            XH = XO_F // 2  # 1568
            t1 = pool.tile([128, XH], BF16)
            t2 = pool.tile([128, XH // 2 + XH // 4], BF16)
            c2, c3 = XH // 2, XH // 4
            o3 = c2
            pmax = pool.tile([128, 2], F32)
            for pl, ph_ in ((0, 64), (64, 128)):
                nc.vector.tensor_tensor(out=t1[pl:ph_, :],
                                        in0=xob[pl:ph_, 0:XH],
                                        in1=xob[pl:ph_, XH:XO_F], op=OP.max)
                nc.vector.tensor_tensor(out=t2[pl:ph_, 0:c2],
                                        in0=t1[pl:ph_, 0:c2],
                                        in1=t1[pl:ph_, c2:XH], op=OP.max)
                nc.vector.tensor_tensor(out=t2[pl:ph_, o3:o3 + c3],
                                        in0=t2[pl:ph_, 0:c3],
                                        in1=t2[pl:ph_, c3:c2], op=OP.max)
                nc.vector.tensor_reduce(out=pmax[pl:ph_, 0:1],
                                        in_=t2[pl:ph_, o3:o3 + c3],
                                        axis=AX.X, op=OP.max)

            # ---- x path first, end to end: its AR, threshold, qscale.
            # The w column must never delay the x path.
            gmax = pool.tile([128, 2], F32)
            trec = pool.tile([128, 2], F32)
            scales = pool.tile([128, 3], F32)
            nc.gpsimd.partition_all_reduce(gmax[:, 0:1], pmax[:, 0:1],
                                           channels=128,
                                           reduce_op=bass_isa.ReduceOp.max)
            nc.vector.tensor_scalar(out=trec[:, 0:1], in0=gmax[:, 0:1],
                                    scalar1=TREC_MUL, scalar2=TREC_ADD_X,
                                    op0=OP.mult, op1=OP.add)
            nc.vector.reciprocal(scales[:, 0:1], trec[:, 0:1])

            # ---- w path (has slack until the first matmul's lhsT). The
            # reduce writes INTO the x tail-reduce's read region: the WAR
            # dependency pins it after the x tree so it can never stall the
            # DVE mid-tree waiting on the (later-landing) w data. bf16 max
            # is fine for T_w: the EMA damps the rounding by 0.05x.
            nc.vector.tensor_reduce(out=t2[:, o3:o3 + 1], in_=w_sb, axis=AX.X,
                                    op=OP.max, apply_absolute_value=True)
            nc.gpsimd.partition_all_reduce(gmax[:, 1:2], t2[:, o3:o3 + 1],
                                           channels=128,
                                           reduce_op=bass_isa.ReduceOp.max)
            nc.vector.tensor_scalar(out=trec[:, 1:2], in0=gmax[:, 1:2],
                                    scalar1=TREC_MUL, scalar2=TREC_ADD_W,
                                    op0=OP.mult, op1=OP.add)
            nc.vector.reciprocal(scales[:, 1:2], trec[:, 1:2])

            # ---- quantize x in two row-chunks (rows 0:16 feed the ph0
            # matmuls, overlapping quantization of rows 16:30), w between
            # them. step-2 writes bf16 (ints in [-128,256) exact; larger
            # clipped by the min); step-3 runs bf16->bf16 (DVE 2x).
            CH = 16 * PAD  # 480: first-chunk columns (rows 0:16)
            tx = pool.tile([2 * CIN, PAD * PAD], F32)
            rx = pool.tile([2 * CIN, PAD * PAD], BF16)
            qx2f = pool.tile([2 * CIN, PAD * PAD], BF16)
            tw = pool.tile([2 * CIN, W_F], F32)
            rw = pool.tile([2 * CIN, W_F], BF16)
            qw = pool.tile([2 * CIN, W_F], BF16)

            WH = 3 * COUT  # 192: weight cols for groups 0-2
            nc.scalar.activation(tx[:, 0:CH], xbw[:, 0:CH], ACTF.Copy,
                                 bias=MAGIC, scale=scales[:, 0:1])
            nc.scalar.activation(tw[:, 0:WH], w_sb[:, 0:WH], ACTF.Copy,
                                 bias=MAGIC, scale=scales[:, 1:2])
            nc.scalar.activation(tw[:, WH:W_F], w_sb[:, WH:W_F], ACTF.Copy,
                                 bias=MAGIC, scale=scales[:, 1:2])
            nc.scalar.activation(tx[:, CH:PAD * PAD], xbw[:, CH:PAD * PAD],
                                 ACTF.Copy, bias=MAGIC, scale=scales[:, 0:1])
            nc.vector.tensor_scalar(out=rx[:, 0:CH], in0=tx[:, 0:CH],
                                    scalar1=MAGIC, scalar2=-128.0,
                                    op0=OP.subtract, op1=OP.max)
            nc.vector.tensor_scalar(out=qx2f[:, 0:CH], in0=rx[:, 0:CH],
                                    scalar1=127.0, scalar2=None, op0=OP.min)
            for lo, hi in ((0, WH), (WH, W_F)):
                nc.vector.tensor_scalar(out=rw[:, lo:hi], in0=tw[:, lo:hi],
                                        scalar1=MAGIC, scalar2=-128.0,
                                        op0=OP.subtract, op1=OP.max)
                nc.vector.tensor_scalar(out=qw[:, lo:hi], in0=rw[:, lo:hi],
                                        scalar1=127.0, scalar2=None,
                                        op0=OP.min)
            nc.vector.tensor_scalar(out=rx[:, CH:PAD * PAD],
                                    in0=tx[:, CH:PAD * PAD],
                                    scalar1=MAGIC, scalar2=-128.0,
                                    op0=OP.subtract, op1=OP.max)
            nc.vector.tensor_scalar(out=qx2f[:, CH:PAD * PAD],
                                    in0=rx[:, CH:PAD * PAD],
                                    scalar1=127.0, scalar2=None, op0=OP.min)
            qx2 = qx2f[:].rearrange("p (h w) -> p h w", h=PAD)

            # ---- ss for the epilogue (off the critical path)
            nc.vector.tensor_tensor(out=scales[:, 2:3], in0=trec[:, 0:1],
                                    in1=trec[:, 1:2], op=OP.mult)

            # ---- conv: 3 kh-pair groups (K=128) + 3 kh=2 singles (K=64)
            ph0 = psum.tile([COUT, PH], F32)
            ph1 = psum.tile([COUT, PH], F32)
            for half, ph in ((0, ph0), (1, ph1)):
                for g in range(6):
                    if g < 3:  # taps (0,kw) + (1,kw), kw = g
                        kh, kw, kp = 0, g, 2 * CIN
                    else:      # tap (2,kw), kw = g - 3
                        kh, kw, kp = 2, g - 3, CIN
                    lhsT = qw[0:kp, g * COUT:(g + 1) * COUT]
                    r0 = kh + 14 * half
                    nc.tensor.matmul(
                        ph[:], lhsT, qx2[0:kp, r0:r0 + 14, kw:kw + W],
                        start=(g == 0), stop=(g == 5))

            # ---- epilogue: out = psum*ss + bias; outputs partition-split
            # across both HWDGE queues to halve the descriptor tail
            out_sb = pool.tile([COUT, P], F32)
            nc.vector.tensor_scalar(out=out_sb[:, 0:PH], in0=ph0[:],
                                    scalar1=scales[0:COUT, 2:3],
                                    scalar2=bias_sb,
                                    op0=OP.mult, op1=OP.add)
            nc.scalar.dma_start(out=out_d[0:16, 0:PH], in_=out_sb[0:16, 0:PH])
            nc.sync.dma_start(out=out_d[32:48, 0:PH], in_=out_sb[32:48, 0:PH])
            nc.scalar.dma_start(out=out_d[16:32, 0:PH],
                                in_=out_sb[16:32, 0:PH])
            nc.sync.dma_start(out=out_d[48:64, 0:PH], in_=out_sb[48:64, 0:PH])
            nc.vector.tensor_scalar(out=out_sb[:, PH:P], in0=ph1[:],
                                    scalar1=scales[0:COUT, 2:3],
                                    scalar2=bias_sb,
                                    op0=OP.mult, op1=OP.add)
            nc.scalar.dma_start(out=out_d[0:16, PH:P], in_=out_sb[0:16, PH:P])
            nc.sync.dma_start(out=out_d[32:48, PH:P], in_=out_sb[32:48, PH:P])
            nc.scalar.dma_start(out=out_d[16:32, PH:P],
                                in_=out_sb[16:32, PH:P])
            nc.sync.dma_start(out=out_d[48:64, PH:P], in_=out_sb[48:64, PH:P])

    nc.compile()
    return nc


_NC = None


def _get_nc():
    global _NC
    if _NC is None:
        _NC = _build()
    return _NC


def _prep_in_maps(x, weight, bias):
    x = np.ascontiguousarray(x, dtype=np.float32)
    bias = np.asarray(bias, dtype=np.float32)
    weight = np.asarray(weight, dtype=np.float32)
    # xbw [128, 1291]: padded-image block + wpair, both pre-shifted
    base = np.zeros((N_CORES, CIN, 940), dtype=np.float32)
    base[:, :, :PAD * PAD].reshape(N_CORES, CIN, PAD, PAD)[
        :, :, 1:1 + H, 1:1 + W] = x.reshape(N_CORES, CIN, H, W)
    xbw = np.zeros((N_CORES, 2 * CIN, XBW_F), dtype=np.float32)
    xbw[:, 0:CIN, 0:XB_F] = base[:, :, 0:XB_F]
    xbw[:, CIN:, 0:XB_F] = base[:, :, PAD:PAD + XB_F]
    xbw[:, :, 904] = np.tile(bias, 2)[None, :]
    wt = np.transpose(weight, (1, 2, 3, 0))  # [Cin, kh, kw, Cout]
    for g in range(3):
        xbw[:, 0:CIN, XB_F + g * COUT:XB_F + (g + 1) * COUT] = wt[:, 0, g, :]
        xbw[:, CIN:, XB_F + g * COUT:XB_F + (g + 1) * COUT] = wt[:, 1, g, :]
        xbw[:, 0:CIN, XB_F + (3 + g) * COUT:XB_F + (4 + g) * COUT] = \
            wt[:, 2, g, :]
    # xob [128, 3136] bf16: |x| of the full batch (absmax-only replica;
    # magnitudes so the on-device max tree needs no abs support)
    xob = np.ascontiguousarray(
        np.abs(x).reshape(128, XO_F).astype(ml_dtypes.bfloat16))
    in_maps = []
    for b in range(N_CORES):
        in_maps.append({
            "xbw": xbw[b],
            "xob": xob,
        })
    return in_maps


def _check_lut(lut):
    idx = np.arange(-128, 128, dtype=np.float32)
    expect = np.outer(idx, idx)
    if not np.array_equal(np.asarray(lut, dtype=np.float32), expect):
        raise ValueError(
            "lut is not the exact int8 product table; this kernel's PE-matmul "
            "formulation only applies to the exact-product LUT.")


def kernel(x, weight, bias, lut):
    _check_lut(lut)
    nc = _get_nc()
    in_maps = _prep_in_maps(np.asarray(x), np.asarray(weight), np.asarray(bias))
    res = run_bass_kernel_spmd(nc, in_maps, core_ids=list(range(N_CORES)))
    out = np.empty((N_CORES, COUT, H, W), dtype=np.float32)
    for b in range(N_CORES):
        out[b] = res.results[b]["out"].reshape(COUT, H, W)
    return out


# revision 20
# speedup vs baseline: 1.0923x; 1.0923x over previous
"""Trainium2 Bass kernel for LUT-based int8-quantized 3x3 conv (ApproxTorch baseline).

Problem: y = conv2d(quant(x), quant(w)) summed via a 256x256 LUT of int8
products, rescaled by (T_f/127)*(T_w/127) + bias, where T_f/T_w are EMA
thresholds updated with the *global* absmax of x / w before the conv.

The LUT staged by setup_inputs() is the exact signed-product table
lut[a+128, b+128] = a*b, so the LUT-gather-sum is an integer matmul; int8
values are exact in bf16 and accumulate exactly in fp32 PSUM, so the PE
array reproduces the reference. We verify the product-table property on
the host and refuse to run otherwise.

Sharding: data-parallel over batch (B=8 -> 1 image/core). The global absmax
of x needs all 8 images on every core: a bf16 replica of the full batch is
loaded per core (absmax only; bf16 rounding of the max costs ~6e-4 output
rel err vs the 2e-2 gate) and the own image in fp32 for exact quantization.
This avoids the ~20us mesh-AllReduce latency floor entirely.

Measured HW facts driving the layout (from NTFF traces):
 - DMA queues are descriptor-dispatch-bound (~27ns/desc scalar HWDGE,
   ~35ns sync, ~42ns gpsimd SWDGE) -> minimize descriptor count: every
   tensor is loaded as [64, long-row] partition halves on its own queue.
 - DVE TENSOR_REDUCE is ~1.09 cyc/elem at any dtype; 16-bit elementwise
   TT/TS ops get the 2x mode (0.63) -> reduce the bf16 batch replica with
   a TT-max tree (halving each level), not a flat reduce.
 - 8-bit DVE ops get no fast mode, so fp8 only shrinks bytes (irrelevant
   when descriptor-bound) -> bf16 replica.

Layout:
 - xbw [128, 1291] f32 (5164B rows): cols 0:900 padded own image (top =
   rows 0:30 of the 30x30 pad-1 image, bottom = the same shifted one
   padded row, so one [128,14,28] moving AP feeds tap (kh,kw) from the
   top half and (kh+1,kw) from the bottom), col 904 bias, cols 907:1291
   wpair (pair groups g0-2 as top/bottom taps (0,g)/(1,g); kh=2 singles
   g3-5 in the top half).
 - xob8 [128, 3136] bf16 (6272B rows): the full batch, absmax only.

Queues: scalar: xob8 top, then xbw bottom; sync: xbw top; gpsimd: xob8
bottom. Outputs are partition-split across scalar+sync.

Pipeline: TT-max tree (top half early, bottom on landing) -> per-partition
max -> gpsimd partition all-reduce -> trec = am*(0.05/127) + 0.95*T/127 ->
qscale = 1/trec (DVE reciprocal) -> quantize via the 1.5*2^23 RNE trick
(ACT Copy(v*qs+MAGIC) -> DVE (t-MAGIC,max -128) -> (min 127) -> bf16) ->
12 matmuls (2 PSUM banks x 6 groups) -> out = psum*(trec_x*trec_w) + bias.
"""

import os
import sys

import numpy as np

for _p in ("/opt/trn_rl_repo", "/root/.axon_site", "/root/.axon_site/_ro/trn_rl_repo",
           "/root/.axon_site/_ro/pypackages"):
    if os.path.isdir(_p) and _p not in sys.path:
        sys.path.append(_p)

import ml_dtypes  # noqa: E402

from concourse import bacc, bass, bass_isa, mybir, tile  # noqa: E402
from concourse.bass_utils import run_bass_kernel_spmd  # noqa: E402

F32 = mybir.dt.float32
BF16 = mybir.dt.bfloat16
AX = mybir.AxisListType
OP = mybir.AluOpType
ACTF = mybir.ActivationFunctionType

N_CORES = 8
CIN = 64
COUT = 64
H = W = 28
P = H * W            # 784 pixels
PH = P // 2          # 392 per PSUM bank (14 output rows)
PAD = 30             # padded spatial edge
XB_F = 907           # padded image block: 900 image + 4 zeros + bias + 2 spare
W_F = 384            # wpair cols: 3 pair groups + 3 kh=2 singles (top half)
XBW_F = XB_F + W_F   # 1291
XO_F = 8 * CIN * P // 128  # 3136: the full batch as bf16
MAGIC = 12582912.0   # 1.5 * 2**23: fp32 add/sub round-to-nearest-even trick

# trec = absmax*(0.05/127) + 0.95*T_init/127; qscale = 1/trec
TREC_MUL = float(np.float32(0.05) / np.float32(127.0))
TREC_ADD_X = float(np.float32(0.95) * np.float32(3.0) / np.float32(127.0))
TREC_ADD_W = float(np.float32(0.95) * np.float32(0.3) / np.float32(127.0))


def _build():
    nc = bacc.Bacc(
        "TRN2",
        target_bir_lowering=False,
        debug=False,
        enable_asserts=True,
        num_devices=N_CORES,
    )
    xbw_d = nc.dram_tensor("xbw", [2 * CIN, XBW_F], F32, kind="ExternalInput")
    xob_d = nc.dram_tensor("xob", [128, XO_F], BF16, kind="ExternalInput")
    out_d = nc.dram_tensor("out", [COUT, P], F32, kind="ExternalOutput")

    with tile.TileContext(nc) as tc:
        with (
            tc.tile_pool(name="sbuf", bufs=1) as pool,
            tc.tile_pool(name="psum", bufs=1, space="PSUM") as psum,
        ):
            # ---- loads: partition-split halves, one per queue slot, to
            # minimize per-queue descriptor counts (the measured bound).
            xbw = pool.tile([2 * CIN, XBW_F], F32)
            xob = pool.tile([128, XO_F], BF16)
            nc.sync.dma_start(out=xob[0:96, :], in_=xob_d[0:96, :])
            nc.scalar.dma_start(out=xob[96:128, :], in_=xob_d[96:128, :])
            nc.sync.dma_start(out=xbw[0:CIN, :], in_=xbw_d[0:CIN, :])
            nc.scalar.dma_start(out=xbw[CIN:2 * CIN, :],
                                in_=xbw_d[CIN:2 * CIN, :])
            w_sb = xbw[:, XB_F:XBW_F]
            bias_sb = xbw[0:COUT, 904:905]

            # ---- absmax of the batch: bf16 TT-max tree (2x DVE mode).
            # Level 1 runs per partition half so the early half overlaps
            # the other half's DMA. |.| via abs into the tree's level 1.
            XH = XO_F // 2  # 1568
            t1 = pool.tile([128, XH], BF16)
            t2 = pool.tile([128, XH // 2 + XH // 4], BF16)
            c2, c3 = XH // 2, XH // 4
            o3 = c2
            pmax = pool.tile([128, 2], F32)
            for pl, ph_ in ((0, 96), (96, 128)):
                nc.vector.tensor_tensor(out=t1[pl:ph_, :],
                                        in0=xob[pl:ph_, 0:XH],
                                        in1=xob[pl:ph_, XH:XO_F], op=OP.max)
            nc.vector.tensor_tensor(out=t2[:, 0:c2], in0=t1[:, 0:c2],
                                    in1=t1[:, c2:XH], op=OP.max)
            nc.vector.tensor_tensor(out=t2[:, o3:o3 + c3], in0=t2[:, 0:c3],
                                    in1=t2[:, c3:c2], op=OP.max)
            nc.vector.tensor_reduce(out=pmax[:, 0:1], in_=t2[:, o3:o3 + c3],
                                    axis=AX.X, op=OP.max)

            # ---- x path first, end to end: its AR, threshold, qscale.
            # The w column must never delay the x path.
            gmax = pool.tile([128, 2], F32)
            trec = pool.tile([128, 2], F32)
            scales = pool.tile([128, 3], F32)
            nc.gpsimd.partition_all_reduce(gmax[:, 0:1], pmax[:, 0:1],
                                           channels=128,
                                           reduce_op=bass_isa.ReduceOp.max)
            nc.vector.tensor_scalar(out=trec[:, 0:1], in0=gmax[:, 0:1],
                                    scalar1=TREC_MUL, scalar2=TREC_ADD_X,
                                    op0=OP.mult, op1=OP.add)
            nc.vector.reciprocal(scales[:, 0:1], trec[:, 0:1])

            # ---- w path (has slack until the first matmul's lhsT). The
            # reduce writes INTO the x tail-reduce's read region: the WAR
            # dependency pins it after the x tree so it can never stall the
            # DVE mid-tree waiting on the (later-landing) w data. bf16 max
            # is fine for T_w: the EMA damps the rounding by 0.05x.
            nc.vector.tensor_reduce(out=t2[:, o3:o3 + 1], in_=w_sb, axis=AX.X,
                                    op=OP.max, apply_absolute_value=True)
            nc.gpsimd.partition_all_reduce(gmax[:, 1:2], t2[:, o3:o3 + 1],
                                           channels=128,
                                           reduce_op=bass_isa.ReduceOp.max)
            nc.vector.tensor_scalar(out=trec[:, 1:2], in0=gmax[:, 1:2],
                                    scalar1=TREC_MUL, scalar2=TREC_ADD_W,
                                    op0=OP.mult, op1=OP.add)
            nc.vector.reciprocal(scales[:, 1:2], trec[:, 1:2])

            # ---- quantize x in two row-chunks (rows 0:16 feed the ph0
            # matmuls, overlapping quantization of rows 16:30), w between
            # them. step-2 writes bf16 (ints in [-128,256) exact; larger
            # clipped by the min); step-3 runs bf16->bf16 (DVE 2x).
            CH = 16 * PAD  # 480: first-chunk columns (rows 0:16)
            tx = pool.tile([2 * CIN, PAD * PAD], F32)
            rx = pool.tile([2 * CIN, PAD * PAD], BF16)
            qx2f = pool.tile([2 * CIN, PAD * PAD], BF16)
            tw = pool.tile([2 * CIN, W_F], F32)
            rw = pool.tile([2 * CIN, W_F], BF16)
            qw = pool.tile([2 * CIN, W_F], BF16)

            WH = 3 * COUT  # 192: weight cols for groups 0-2
            nc.scalar.activation(tx[:, 0:CH], xbw[:, 0:CH], ACTF.Copy,
                                 bias=MAGIC, scale=scales[:, 0:1])
            nc.scalar.activation(tw[:, 0:WH], w_sb[:, 0:WH], ACTF.Copy,
                                 bias=MAGIC, scale=scales[:, 1:2])
            nc.scalar.activation(tw[:, WH:W_F], w_sb[:, WH:W_F], ACTF.Copy,
                                 bias=MAGIC, scale=scales[:, 1:2])
            nc.scalar.activation(tx[:, CH:PAD * PAD], xbw[:, CH:PAD * PAD],
                                 ACTF.Copy, bias=MAGIC, scale=scales[:, 0:1])
            nc.vector.tensor_scalar(out=rx[:, 0:CH], in0=tx[:, 0:CH],
                                    scalar1=MAGIC, scalar2=-128.0,
                                    op0=OP.subtract, op1=OP.max)
            nc.vector.tensor_scalar(out=qx2f[:, 0:CH], in0=rx[:, 0:CH],
                                    scalar1=127.0, scalar2=None, op0=OP.min)
            for lo, hi in ((0, WH), (WH, W_F)):
                nc.vector.tensor_scalar(out=rw[:, lo:hi], in0=tw[:, lo:hi],
                                        scalar1=MAGIC, scalar2=-128.0,
                                        op0=OP.subtract, op1=OP.max)
                nc.vector.tensor_scalar(out=qw[:, lo:hi], in0=rw[:, lo:hi],
                                        scalar1=127.0, scalar2=None,
                                        op0=OP.min)
            nc.vector.tensor_scalar(out=rx[:, CH:PAD * PAD],
                                    in0=tx[:, CH:PAD * PAD],
                                    scalar1=MAGIC, scalar2=-128.0,
                                    op0=OP.subtract, op1=OP.max)
            nc.vector.tensor_scalar(out=qx2f[:, CH:PAD * PAD],
                                    in0=rx[:, CH:PAD * PAD],
                                    scalar1=127.0, scalar2=None, op0=OP.min)
            qx2 = qx2f[:].rearrange("p (h w) -> p h w", h=PAD)

            # ---- ss for the epilogue (off the critical path)
            nc.vector.tensor_tensor(out=scales[:, 2:3], in0=trec[:, 0:1],
                                    in1=trec[:, 1:2], op=OP.mult)

            # ---- conv: 3 kh-pair groups (K=128) + 3 kh=2 singles (K=64)
            ph0 = psum.tile([COUT, PH], F32)
            ph1 = psum.tile([COUT, PH], F32)
            for half, ph in ((0, ph0), (1, ph1)):
                for g in range(6):
                    if g < 3:  # taps (0,kw) + (1,kw), kw = g
                        kh, kw, kp = 0, g, 2 * CIN
                    else:      # tap (2,kw), kw = g - 3
                        kh, kw, kp = 2, g - 3, CIN
                    lhsT = qw[0:kp, g * COUT:(g + 1) * COUT]
                    r0 = kh + 14 * half
                    nc.tensor.matmul(
                        ph[:], lhsT, qx2[0:kp, r0:r0 + 14, kw:kw + W],
                        start=(g == 0), stop=(g == 5))

            # ---- epilogue: out = psum*ss + bias; outputs partition-split
            # across both HWDGE queues to halve the descriptor tail
            out_sb = pool.tile([COUT, P], F32)
            nc.vector.tensor_scalar(out=out_sb[:, 0:PH], in0=ph0[:],
                                    scalar1=scales[0:COUT, 2:3],
                                    scalar2=bias_sb,
                                    op0=OP.mult, op1=OP.add)

            nc.vector.tensor_scalar(out=out_sb[:, PH:P], in0=ph1[:],
                                    scalar1=scales[0:COUT, 2:3],
                                    scalar2=bias_sb,
                                    op0=OP.mult, op1=OP.add)
            nc.scalar.dma_start(out=out_d[0:32, :], in_=out_sb[0:32, :])
            nc.sync.dma_start(out=out_d[32:64, :], in_=out_sb[32:64, :])

    nc.compile()
    return nc


_NC = None


def _get_nc():
    global _NC
    if _NC is None:
        _NC = _build()
    return _NC


def _prep_in_maps(x, weight, bias):
    x = np.ascontiguousarray(x, dtype=np.float32)
    bias = np.asarray(bias, dtype=np.float32)
    weight = np.asarray(weight, dtype=np.float32)
    # xbw [128, 1291]: padded-image block + wpair, both pre-shifted
    base = np.zeros((N_CORES, CIN, 940), dtype=np.float32)
    base[:, :, :PAD * PAD].reshape(N_CORES, CIN, PAD, PAD)[
        :, :, 1:1 + H, 1:1 + W] = x.reshape(N_CORES, CIN, H, W)
    xbw = np.zeros((N_CORES, 2 * CIN, XBW_F), dtype=np.float32)
    xbw[:, 0:CIN, 0:XB_F] = base[:, :, 0:XB_F]
    xbw[:, CIN:, 0:XB_F] = base[:, :, PAD:PAD + XB_F]
    xbw[:, :, 904] = np.tile(bias, 2)[None, :]
    wt = np.transpose(weight, (1, 2, 3, 0))  # [Cin, kh, kw, Cout]
    for g in range(3):
        xbw[:, 0:CIN, XB_F + g * COUT:XB_F + (g + 1) * COUT] = wt[:, 0, g, :]
        xbw[:, CIN:, XB_F + g * COUT:XB_F + (g + 1) * COUT] = wt[:, 1, g, :]
        xbw[:, 0:CIN, XB_F + (3 + g) * COUT:XB_F + (4 + g) * COUT] = \
            wt[:, 2, g, :]
    # xob [128, 3136] bf16: |x| of the full batch (absmax-only replica;
    # magnitudes so the on-device max tree needs no abs support)
    xob = np.ascontiguousarray(
        np.abs(x).reshape(128, XO_F).astype(ml_dtypes.bfloat16))
    in_maps = []
    for b in range(N_CORES):
        in_maps.append({
            "xbw": xbw[b],
            "xob": xob,
        })
    return in_maps


def _check_lut(lut):
    idx = np.arange(-128, 128, dtype=np.float32)
    expect = np.outer(idx, idx)
    if not np.array_equal(np.asarray(lut, dtype=np.float32), expect):
        raise ValueError(
            "lut is not the exact int8 product table; this kernel's PE-matmul "
            "formulation only applies to the exact-product LUT.")


def kernel(x, weight, bias, lut):
    _check_lut(lut)
    nc = _get_nc()
    in_maps = _prep_in_maps(np.asarray(x), np.asarray(weight), np.asarray(bias))
    res = run_bass_kernel_spmd(nc, in_maps, core_ids=list(range(N_CORES)))
    out = np.empty((N_CORES, COUT, H, W), dtype=np.float32)
    for b in range(N_CORES):
        out[b] = res.results[b]["out"].reshape(COUT, H, W)
    return out
